# revision 1
# baseline (speedup 1.0000x reference)
"""Trainium2 Bass kernel for a dense transformer encoder layer.

Reference semantics (B=2, S=2048, D=1024, H=16, DH=64, HID=4096):
    q = einsum('bsd,hde->bhse', x, Wq) + bq          (q == k == v, source bug)
    prob = softmax(q @ q^T / sqrt(DH))
    attn = concat_heads(prob @ q)
    x1 = LN(x + attn);  ff = relu(x1 @ W1 + b1) @ W2 + b2;  out = LN(x1 + ff)

Sharding: 8 cores, core c -> batch b=c//4, token quarter t=c%4.  Each core
computes q for the full sequence of its batch (replicated inside the 4-core
group -> zero collectives), then attention + FFN for its own 512 tokens.
The host rotates each core's copy of x[b] so its quarter lands at rows 0:512
(attention is permutation-equivariant over keys), and reassembles quarters.

On-chip dataflow is bf16 matmul / f32 accumulate. Layout flips (x->xT,
qT->q-natural(+ones column for softmax denominators), uvT->attn, x1->x1T,
ffT->ff) go through DRAM round trips using the HWDGE xbar transpose.
Scratch tensors are split into head/d halves so the post-attention and
post-FFN epilogues start while the second half is still computing; epilogue
pools are opened before the attention/FFN pools so their SBUF regions are
disjoint (stack reuse would otherwise serialize the phases).
"""

import numpy as np

import concourse.bacc as bacc
import concourse.mybir as mybir
from concourse import tile
from concourse.bass_utils import run_bass_kernel_spmd

dt = mybir.dt
AF = mybir.ActivationFunctionType
ALU = mybir.AluOpType

B, S, D = 2, 2048, 1024
H, DH, HID = 16, 64, 256 * 16
SQ = S // 4            # tokens per core
NCORES = 8
EPS = 1e-5
F32, BF16 = dt.float32, dt.bfloat16

_BUILD_CACHE = {}


def _build(apply_affine: bool):
    if apply_affine in _BUILD_CACHE:
        return _BUILD_CACHE[apply_affine]

    nc = bacc.Bacc("TRN2", target_bir_lowering=False, debug=False,
                   num_devices=NCORES)

    x_bf = nc.dram_tensor("x_bf", [S, D], BF16, kind="ExternalInput").ap()
    x_q = nc.dram_tensor("x_q", [SQ, D], F32, kind="ExternalInput").ap()
    wq = nc.dram_tensor("wq", [D, D], BF16, kind="ExternalInput").ap()
    bq_r = nc.dram_tensor("bq_r", [128, 8], F32, kind="ExternalInput").ap()
    w1 = nc.dram_tensor("w1", [D, HID], BF16, kind="ExternalInput").ap()
    b1_r = nc.dram_tensor("b1_r", [128, 32], F32, kind="ExternalInput").ap()
    w2 = nc.dram_tensor("w2", [HID, D], BF16, kind="ExternalInput").ap()
    b2_r = nc.dram_tensor("b2_r", [128, 8], F32, kind="ExternalInput").ap()
    if apply_affine:
        g1d = nc.dram_tensor("g1d", [128, D], F32, kind="ExternalInput").ap()
        be1d = nc.dram_tensor("be1d", [128, D], F32, kind="ExternalInput").ap()
        g2d = nc.dram_tensor("g2d", [128, D], F32, kind="ExternalInput").ap()
        be2d = nc.dram_tensor("be2d", [128, D], F32, kind="ExternalInput").ap()
    out_q = nc.dram_tensor("out_q", [SQ, D], F32, kind="ExternalOutput").ap()

    with tile.TileContext(nc) as tc:
        with (
            tc.tile_pool(name="dram", bufs=1, space="DRAM") as dpool,
            tc.tile_pool(name="const", bufs=1) as cpool,
        ):
            q_d = dpool.tile([H * 80, S], BF16)
            uv_dl = dpool.tile([D // 2, SQ], BF16)
            uv_dh = dpool.tile([D // 2, SQ], BF16)
            cs_dl = dpool.tile([4, 1024], F32)
            cs_dh = dpool.tile([4, 1024], F32)
            ff_dl = dpool.tile([D // 2, SQ], BF16)
            ff_dh = dpool.tile([D // 2, SQ], BF16)
            cs16l = cs_dl.rearrange("a (j s) -> (a j) s", j=2)
            cs16h = cs_dh.rearrange("a (j s) -> (a j) s", j=2)

            bq_sb = cpool.tile([128, 8], F32)
            nc.scalar.dma_start(bq_sb[:], bq_r[:])
            b1_sb = cpool.tile([128, 32], F32)
            nc.scalar.dma_start(b1_sb[:], b1_r[:])
            b2_sb = cpool.tile([128, 8], F32)
            nc.scalar.dma_start(b2_sb[:], b2_r[:])
            if apply_affine:
                g1_sb = cpool.tile([128, D], F32)
                nc.scalar.dma_start(g1_sb[:], g1d[:])
                be1_sb = cpool.tile([128, D], F32)
                nc.scalar.dma_start(be1_sb[:], be1d[:])
                g2_sb = cpool.tile([128, D], F32)
                nc.scalar.dma_start(g2_sb[:], g2d[:])
                be2_sb = cpool.tile([128, D], F32)
                nc.scalar.dma_start(be2_sb[:], be2d[:])

            eps_sb = cpool.tile([128, 1], F32)
            nc.vector.memset(eps_sb[:], EPS)

            # 128x128 bf16 identity for PE-mode transposes
            col_i = cpool.tile([128, 128], F32)
            nc.gpsimd.iota(col_i[:], [[1, 128]], channel_multiplier=0,
                           allow_small_or_imprecise_dtypes=True)
            row_i = cpool.tile([128, 1], F32)
            nc.gpsimd.iota(row_i[:], [[0, 1]], channel_multiplier=1,
                           allow_small_or_imprecise_dtypes=True)
            idn = cpool.tile([128, 128], BF16)
            nc.vector.tensor_scalar(idn[:], col_i[:], row_i[:, 0:1], None,
                                    ALU.is_equal)

            # ones rows of the augmented q (row 64 of every 80-row head block)
            ones_bf = cpool.tile([16, 512], BF16)
            nc.vector.memset(ones_bf[:], 1.0)
            q_d_rows = q_d.rearrange("(h r) s -> h r s", r=80)
            for c4 in range(4):
                nc.scalar.dma_start(
                    q_d_rows[:, 64, c4 * 512:(c4 + 1) * 512], ones_bf[:])

            with (
                tc.tile_pool(name="qT", bufs=1) as qTpool,
                tc.tile_pool(name="ln1", bufs=2) as lpool,
                tc.tile_pool(name="x1f", bufs=1) as x1pool,
                tc.tile_pool(name="ln2", bufs=2) as l2pool,
                tc.tile_pool(name="x1T", bufs=1) as xtp,
            ):
                x1T = [xtp.tile([128, SQ], BF16, tag=f"x1T{k}",
                                name=f"x1T{k}") for k in range(8)]
                qT = [qTpool.tile([128, S], BF16, tag=f"qT{e}", name=f"qT{e}")
                      for e in range(8)]

                # Phase-D input loaders; emitted mid-attention so the first
                # half streams in while heads 8-15 are still computing.
                ln_in = [[None] * 4, [None] * 4]   # [half][sub] -> (rct, ab)
                y1s = []

                def emit_ln1_inputs(hi, subs=range(4)):
                    cs16x = (cs16l, cs16h)[hi]
                    uv_dx = (uv_dl, uv_dh)[hi]
                    for sub in subs:
                        scols = slice(sub * 128, (sub + 1) * 128)
                        ct = lpool.tile([128, 8], F32, tag=f"ct{hi}", bufs=4,
                                        name=f"ct{hi}_{sub}")
                        nc.sync.dma_start(
                            ct[:], cs16x[:, scols].rearrange("h p -> p h"))
                        rct = lpool.tile([128, 8], F32, tag=f"rct{hi}", bufs=4,
                                         name=f"rct{hi}_{sub}")
                        nc.vector.reciprocal(rct[:], ct[:])
                        ab = lpool.tile([128, D // 2], BF16, tag=f"attn{hi}",
                                        bufs=4, name=f"attn{hi}_{sub}")
                        nc.sync.dma_start(ab[:], uv_dx[:, scols],
                                          transpose=True)
                        ln_in[hi][sub] = (rct, ab)
                        if hi == 0:
                            y1 = lpool.tile([128, D], F32, tag="y1", bufs=4,
                                            name=f"y1_{sub}")
                            nc.scalar.dma_start(
                                y1[:], x_q[sub * 128:(sub + 1) * 128, :])
                            y1s.append(y1)

                # ---- Phases B+C interleaved: qproj(e=p) then attention
                # pair p; the next pair's projection fills PE while ACT burns
                # through the softmax exps of the current pair. ----
                with (
                    tc.tile_pool(name="xT", bufs=1) as xTpool,
                    tc.tile_pool(name="wq", bufs=1) as wqpool,
                    tc.tile_pool(name="qa", bufs=1) as qapool,
                    tc.tile_pool(name="att", bufs=4) as apool,
                    tc.tile_pool(name="qps", bufs=2, space="PSUM") as qps,
                    tc.tile_pool(name="scps", bufs=2, space="PSUM") as scps,
                    tc.tile_pool(name="uvps", bufs=2, space="PSUM") as uvps,
                ):
                    xT = []
                    for k in range(8):
                        cols = slice(k * 128, (k + 1) * 128)
                        t = xTpool.tile([128, S], BF16, tag=f"xT{k}")
                        eng = nc.sync if k % 2 == 0 else nc.scalar
                        eng.dma_start(t[:], x_bf[:, cols], transpose=True)
                        xT.append(t)
                    wq_sb = []
                    for k in range(8):
                        t = wqpool.tile([128, D], BF16, tag=f"wq{k}")
                        nc.scalar.dma_start(t[:], wq[k * 128:(k + 1) * 128, :])
                        wq_sb.append(t)

                    for p in range(8):
                        # q projection for head pair p -> qT[p]
                        for n in range(4):
                            ps = qps.tile([128, 512], F32, tag="qps",
                                          name=f"qps{p}_{n}")
                            for k in range(8):
                                nc.tensor.matmul(
                                    ps[:],
                                    wq_sb[k][:, p * 128:(p + 1) * 128],
                                    xT[k][:, n * 512:(n + 1) * 512],
                                    start=(k == 0), stop=(k == 7))
                            nc.vector.tensor_scalar_add(
                                qT[p][:, n * 512:(n + 1) * 512], ps[:],
                                bq_sb[:, p:p + 1])
                        # store into q_d (80-row head blocks; row 64 is ones)
                        for half in range(2):
                            h = 2 * p + half
                            nc.sync.dma_start(
                                q_d[h * 80:h * 80 + 64, :],
                                qT[p][half * 64:half * 64 + 64, :])
                        # natural-layout augmented q for this pair
                        qa_p = []
                        for c in range(16):
                            t = qapool.tile([128, 160], BF16, tag=f"qa{c % 4}",
                                            bufs=8, name=f"qa{p}_{c}")
                            nc.sync.dma_start(
                                t[:],
                                q_d[p * 160:(p + 1) * 160,
                                    c * 128:(c + 1) * 128],
                                transpose=True)
                            qa_p.append(t)

                        # attention for heads 2p, 2p+1
                        uv = [uvps.tile([65, 512], F32, tag="uv",
                                        name=f"uv{p}_{i}") for i in range(2)]
                        prev = None  # (E0, E1, cg)

                        def emit_wv(E0p, E1p, cgp, start, stop):
                            for cc in range(2):
                                c = 2 * cgp + cc
                                for half, Ep in ((0, E0p), (1, E1p)):
                                    nc.tensor.matmul(
                                        uv[half][:],
                                        qa_p[c][:, half * 80:half * 80 + 65],
                                        Ep[:, cc * 512:(cc + 1) * 512],
                                        start=start and cc == 0,
                                        stop=stop and cc == 1)

                        for cg in range(8):
                            sc = [scps.tile([128, 1024], F32, tag="sc",
                                            name=f"sc{p}_{cg}_{i}")
                                  for i in range(2)]
                            for cc in range(2):
                                c = 2 * cg + cc
                                for half in range(2):
                                    nc.tensor.matmul(
                                        sc[half][:, cc * 512:(cc + 1) * 512],
                                        qT[p][half * 64:half * 64 + 64,
                                              c * 128:(c + 1) * 128],
                                        qT[p][half * 64:half * 64 + 64,
                                              0:512],
                                        start=True, stop=True)
                            E = [apool.tile([128, 1024], BF16, tag="E",
                                            bufs=3, name=f"E{p}_{cg}_{i}")
                                 for i in range(2)]
                            for half in range(2):
                                nc.scalar.activation(
                                    E[half][:], sc[half][:], AF.Exp,
                                    scale=0.125)
                            if prev is not None:
                                emit_wv(prev[0], prev[1], prev[2],
                                        prev[2] == 0, False)
                            prev = (E[0], E[1], cg)
                        emit_wv(prev[0], prev[1], prev[2], False, True)

                        # unnormalized head outputs + softmax denominators
                        uv_dst = uv_dl if p < 4 else uv_dh
                        cs_dst = cs_dl if p < 4 else cs_dh
                        pp = p % 4
                        for half in range(2):
                            h = 2 * pp + half
                            uvT_sb = apool.tile([64, 512], BF16, tag="uvT")
                            nc.vector.tensor_copy(uvT_sb[:],
                                                  uv[half][0:64, :])
                            nc.sync.dma_start(
                                uv_dst[h * 64:(h + 1) * 64, :], uvT_sb[:])
                            cs_sb = apool.tile([65, 512], F32, tag="cs",
                                               bufs=2,
                                               name=f"cs{p}_{half}")
                            nc.vector.tensor_copy(cs_sb[64:65, :],
                                                  uv[half][64:65, :])
                            nc.sync.dma_start(
                                cs_dst[pp:pp + 1,
                                       half * 512:(half + 1) * 512],
                                cs_sb[64:65, :])
                        if p >= 3:
                            # stream the heads-0..7 epilogue one token-sub at
                            # a time behind pairs 3-6 to spread DVE load
                            sub = p - 3
                            if sub < 4:
                                emit_ln1_inputs(0, [sub])
                                rct, ab = ln_in[0][sub]
                                for hh in range(8):
                                    sl = slice(hh * 64, (hh + 1) * 64)
                                    nc.vector.scalar_tensor_tensor(
                                        y1s[sub][:, sl],
                                        ab[:, hh * 64:(hh + 1) * 64],
                                        rct[:, hh:hh + 1],
                                        y1s[sub][:, sl], ALU.mult, ALU.add)
                    emit_ln1_inputs(1)

                # ---- Phase D: heads 8-15 residual + LN1 ----
                for sub in range(4):
                    rct, ab = ln_in[1][sub]
                    for hh in range(8):
                        h = 8 + hh
                        sl = slice(h * 64, (h + 1) * 64)
                        nc.vector.scalar_tensor_tensor(
                            y1s[sub][:, sl],
                            ab[:, hh * 64:(hh + 1) * 64],
                            rct[:, hh:hh + 1],
                            y1s[sub][:, sl], ALU.mult, ALU.add)
                x1_f32 = []
                with (
                    tc.tile_pool(name="lnps", bufs=2, space="PSUM") as lnps,
                    tc.tile_pool(name="tps", bufs=4, space="PSUM") as tps,
                ):
                    for sub in range(4):
                        x1 = x1pool.tile([128, D], F32, tag=f"x1_{sub}",
                                         name=f"x1_{sub}")
                        _layer_norm(nc, lpool, lnps, y1s[sub], x1, eps_sb,
                                    (g1_sb, be1_sb) if apply_affine
                                    else None)
                        x1_f32.append(x1)
                        x1bf = lpool.tile([128, D], BF16, tag="x1bf",
                                          bufs=2)
                        nc.scalar.copy(x1bf[:], x1[:])
                        for k in range(8):
                            pst = tps.tile([128, 128], BF16, tag="tps",
                                           name=f"tps{sub}_{k}")
                            nc.tensor.transpose(
                                pst[:], x1bf[:, k * 128:(k + 1) * 128],
                                idn[:])
                            nc.scalar.copy(
                                x1T[k][:, sub * 128:(sub + 1) * 128],
                                pst[:])

                ffb_lo = [None] * 4
                y2s = []

                def emit_ffb(hi, store):
                    ff_dx = (ff_dl, ff_dh)[hi]
                    for sub in range(4):
                        scols = slice(sub * 128, (sub + 1) * 128)
                        ffb = l2pool.tile([128, D // 2], BF16,
                                          tag=f"ffb{hi}", bufs=4,
                                          name=f"ffb{hi}_{sub}")
                        nc.sync.dma_start(ffb[:], ff_dx[:, scols],
                                          transpose=True)
                        store[sub] = ffb

                # ---- Phase E: FFN ----
                with (
                    tc.tile_pool(name="h1", bufs=1) as h1pool,
                    tc.tile_pool(name="wstr", bufs=3) as wpool,
                    tc.tile_pool(name="fps", bufs=4, space="PSUM") as fps,
                ):
                    h1t = []
                    for j in range(32):
                        w1t = wpool.tile([128, 8, 128], BF16, tag="w1t",
                                         bufs=5)
                        nc.scalar.dma_start(
                            w1t[:],
                            w1[:, j * 128:(j + 1) * 128]
                            .rearrange("(k p) c -> p k c", p=128))
                        ps = fps.tile([128, 512], F32, tag="fps")
                        for k in range(8):
                            nc.tensor.matmul(ps[:], w1t[:, k, :], x1T[k][:],
                                             start=(k == 0), stop=(k == 7))
                        ht = h1pool.tile([128, SQ], BF16, tag=f"h1_{j}")
                        nc.vector.tensor_scalar(
                            ht[:], ps[:], b1_sb[:, j:j + 1], 0.0,
                            ALU.add, ALU.max)
                        h1t.append(ht)
                    w2r = w2.rearrange("(j p) c -> j p c", p=128)
                    for i in range(8):
                        w2h = []
                        for hh in range(2):
                            t = wpool.tile([128, 16, 128], BF16, tag="w2t",
                                           bufs=4, name=f"w2t{i}_{hh}")
                            nc.scalar.dma_start(
                                t[:],
                                w2r[hh * 16:(hh + 1) * 16, :,
                                    i * 128:(i + 1) * 128]
                                .rearrange("j p c -> p j c"))
                            w2h.append(t)
                        ps = fps.tile([128, 512], F32, tag="fps")
                        for j in range(32):
                            nc.tensor.matmul(ps[:], w2h[j // 16][:, j % 16, :],
                                             h1t[j][:],
                                             start=(j == 0), stop=(j == 31))
                        fft = wpool.tile([128, SQ], BF16, tag="fft", bufs=2)
                        nc.vector.tensor_scalar_add(fft[:], ps[:],
                                                    b2_sb[:, i:i + 1])
                        ff_dst = ff_dl if i < 4 else ff_dh
                        nc.sync.dma_start(
                            ff_dst[(i % 4) * 128:(i % 4 + 1) * 128, :],
                            fft[:])
                        if i == 3:
                            emit_ffb(0, ffb_lo)
                            for sub in range(4):
                                y2 = l2pool.tile([128, D], F32, tag="y2",
                                                 bufs=4, name=f"y2_{sub}")
                                nc.gpsimd.tensor_add(
                                    y2[:, 0:512],
                                    x1_f32[sub][:, 0:512],
                                    ffb_lo[sub][:])
                                y2s.append(y2)

                # ---- Phase F: residual + LN2 + output ----
                ffb_hi = [None] * 4
                emit_ffb(1, ffb_hi)
                with tc.tile_pool(name="l2ps", bufs=2,
                                  space="PSUM") as l2ps:
                    for sub in range(4):
                        y2 = y2s[sub]
                        nc.gpsimd.tensor_add(
                            y2[:, 512:1024], x1_f32[sub][:, 512:1024],
                            ffb_hi[sub][:])
                        x2 = l2pool.tile([128, D], F32, tag="x2")
                        _layer_norm(nc, lpool, l2ps, y2, x2, eps_sb,
                                    (g2_sb, be2_sb) if apply_affine
                                    else None)
                        nc.sync.dma_start(
                            out_q[sub * 128:(sub + 1) * 128, :], x2[:])

    nc.compile()
    _BUILD_CACHE[apply_affine] = nc
    return nc


def _layer_norm(nc, pool, pspool, y, out, eps_sb, affine):
    """out = (y - mean(y)) * rsqrt(var(y) + EPS) [* g + b], free-dim D."""
    s1 = pool.tile([128, 1], F32, tag="ln_s1")
    nc.vector.reduce_sum(s1[:], y[:], axis=mybir.AxisListType.X)
    mean = pool.tile([128, 1], F32, tag="ln_mean")
    nc.vector.tensor_scalar_mul(mean[:], s1[:], 1.0 / D)
    cen = pool.tile([128, D], F32, tag="ln_cen")
    nc.vector.tensor_scalar_sub(cen[:], y[:], mean[:])
    scr = pspool.tile([128, D], F32, tag="ln_scr")
    var = pool.tile([128, 1], F32, tag="ln_var")
    nc.scalar.activation(scr[:], cen[:], AF.Square, accum_out=var[:])
    std = pool.tile([128, 1], F32, tag="ln_std")
    nc.scalar.activation(std[:], var[:], AF.Sqrt, bias=eps_sb[:, 0:1],
                         scale=1.0 / D)
    rstd = pool.tile([128, 1], F32, tag="ln_rstd")
    nc.vector.reciprocal(rstd[:], std[:])
    if affine is None:
        nc.vector.tensor_scalar_mul(out[:], cen[:], rstd[:])
    else:
        g_sb, b_sb = affine
        nc.vector.scalar_tensor_tensor(
            out[:], cen[:], rstd[:], g_sb[:], ALU.mult, ALU.mult)
        nc.vector.tensor_add(out[:], out[:], b_sb[:])


def kernel(x, Wq, bq, ln1_g, ln1_b, W1, b1, W2, b2, ln2_g, ln2_b):
    x = np.asarray(x, np.float32)
    bf = dt.np(BF16)
    trivial = (np.all(ln1_g == 1) and np.all(ln1_b == 0)
               and np.all(ln2_g == 1) and np.all(ln2_b == 0))
    nc = _build(apply_affine=not trivial)

    base = {
        "wq": np.ascontiguousarray(
            np.asarray(Wq, np.float32).transpose(1, 0, 2).reshape(D, D)
        ).astype(bf),
        "bq_r": np.ascontiguousarray(
            np.asarray(bq, np.float32).reshape(8, 128).T),
        "w1": np.asarray(W1, np.float32).astype(bf),
        "b1_r": np.ascontiguousarray(
            np.asarray(b1, np.float32).reshape(32, 128).T),
        "w2": np.asarray(W2, np.float32).astype(bf),
        "b2_r": np.ascontiguousarray(
            np.asarray(b2, np.float32).reshape(8, 128).T),
    }
    if not trivial:
        for name, v in (("g1d", ln1_g), ("be1d", ln1_b),
                        ("g2d", ln2_g), ("be2d", ln2_b)):
            base[name] = np.ascontiguousarray(
                np.broadcast_to(np.asarray(v, np.float32), (128, D)))

    in_maps = []
    for c in range(NCORES):
        b, t = divmod(c, 4)
        xb = np.concatenate([x[b, t * SQ:], x[b, :t * SQ]], axis=0)
        in_maps.append({
            **base,
            "x_bf": np.ascontiguousarray(xb).astype(bf),
            "x_q": np.ascontiguousarray(x[b, t * SQ:(t + 1) * SQ]),
        })

    import os
    trace = bool(int(os.environ.get("KERNEL_TRACE", "0")))
    kw = {}
    if trace:
        kw = dict(trace=True,
                  tmpdir=os.environ.get("KERNEL_TRACE_DIR") or None)
    res = run_bass_kernel_spmd(nc, in_maps, core_ids=list(range(NCORES)),
                               **kw)
    if trace:
        print(f"HW exec time: {res.exec_time_ns} ns  "
              f"(mean {res.mean_exec_time_ns}, "
              f"max core {res.max_exec_time_core_id})")
    out = np.empty((B, S, D), np.float32)
    for c in range(NCORES):
        b, t = divmod(c, 4)
        out[b, t * SQ:(t + 1) * SQ] = res.results[c]["out_q"]
    return out



# revision 17
# speedup vs baseline: 1.6635x; 1.6635x over previous
"""Trainium2 Bass kernel for a dense transformer encoder layer.

Reference semantics (B=2, S=2048, D=1024, H=16, DH=64, HID=4096):
    q = einsum('bsd,hde->bhse', x, Wq) + bq          (q == k == v, source bug)
    prob = softmax(q @ q^T / sqrt(DH))
    attn = concat_heads(prob @ q)
    x1 = LN(x + attn);  ff = relu(x1 @ W1 + b1) @ W2 + b2;  out = LN(x1 + ff)

Sharding: 8 cores, core c -> batch b=c//4, token quarter t=c%4.  Each core
computes q for the full sequence of its batch (replicated inside the 4-core
group -> zero collectives), then attention + FFN for its own 512 tokens.
The host rotates each core's copy of x[b] so its quarter lands at rows 0:512
(attention is permutation-equivariant over keys), and reassembles quarters.

Precision strategy: all five matmul groups run as fp8(e4m3) DoubleRow
(qproj / scores / wv / FFN-w1 / FFN-w2), accumulating in f32 PSUM.  W1 is
pre-scaled by 32 and W2 by 64 so their entries sit in e4m3's normal range;
the combined 1/2048 is folded into the final bias-add.  Softmax weights are
unnormalized E = exp(s/8 - 2) stored in fp8: the ACT engine computes exact
exp for its share of tiles while the DVE computes a Schraudolph-style
bit-construction (s*log2e + B rounded to int8 IS the e4m3 bit pattern of
~exp(s/8-2)); denominators come from an extra ones-column matmul and divide
through after the fact, so the uniform scale/approximation bias cancels.

Layout tricks: the host uploads x already transposed and fold-paired for
DoubleRow, so the only on-device transposes are batched 16x128-tile XBAR
DMA transposes (qa, x1T, ffb) -- each one instruction for a whole [128,
2048] operand, keeping the HWDGE issue count (~625ns each, exclusive) low.
Scores contract over DH=64 folded to [32,2] partitions via 4 SBUF->SBUF
partition-moving DMAs per head pair.
"""

import numpy as np

import concourse.bacc as bacc
import concourse.mybir as mybir
from concourse import tile
from concourse.bass_utils import run_bass_kernel_spmd
from concourse.tile_rust import add_dep_helper

dt = mybir.dt
AF = mybir.ActivationFunctionType
ALU = mybir.AluOpType
MM_DR = mybir.MatmulPerfMode.DoubleRow

B, S, D = 2, 2048, 1024
H, DH, HID = 16, 64, 4096
SQ = S // 4            # tokens per core
NCORES = 8
EPS = 1e-5
F32, BF16, F8, U8 = dt.float32, dt.bfloat16, dt.float8e4, dt.uint8

W1S, W2S = 32.0, 64.0          # host-side fp8 weight scales
FFS = 1.0 / (W1S * W2S)        # folded back in the last FFN bias-add
LOG2E = 1.4426950408889634
EXP_B8 = 32.58                 # e4m3 bit bias: 55.66 - 8*2*log2(e), so the
                               # bit-trick matches the ACT tiles' exp(s/8-2)
EXP_BIAS = -2.0                # matching shift on the exact-exp (ACT) tiles

# exp units (g, hh) handed to the DVE bit-trick; the rest go to ACT exact
# exp.  Interleaved through the pair so ACT and DVE run concurrently.
import os
EXP_DVE_UNITS = (set() if os.environ.get("NO_DVE_EXP") else
                 {(0, 1), (2, 0), (3, 1), (5, 0), (6, 1)})

_BUILD_CACHE = {}


def _build(apply_affine: bool):
    if apply_affine in _BUILD_CACHE:
        return _BUILD_CACHE[apply_affine]

    nc = bacc.Bacc("TRN2", target_bir_lowering=False, debug=False,
                   num_devices=NCORES)

    xt2_d = nc.dram_tensor("xt2", [128, 4 * 2 * S], F8,
                           kind="ExternalInput").ap()
    xq_d = nc.dram_tensor("xq", [SQ, D], BF16, kind="ExternalInput").ap()
    wq_d = nc.dram_tensor("wq2", [128, 4 * 2 * D], F8,
                          kind="ExternalInput").ap()
    bq_d = nc.dram_tensor("bqr", [128, 8], F32, kind="ExternalInput").ap()
    w1_d = nc.dram_tensor("w1p", [128, 8 * 4096], F8,
                          kind="ExternalInput").ap()
    b1_d = nc.dram_tensor("b1r", [128, 32], F32, kind="ExternalInput").ap()
    w2_d = nc.dram_tensor("w2p", [128, 8 * 4096], F8,
                          kind="ExternalInput").ap()
    b2_d = nc.dram_tensor("b2r", [128, 8], F32, kind="ExternalInput").ap()
    if apply_affine:
        g1_d = nc.dram_tensor("g1d", [128, D], F32, kind="ExternalInput").ap()
        be1_d = nc.dram_tensor("be1d", [128, D], F32,
                               kind="ExternalInput").ap()
        g2_d = nc.dram_tensor("g2d", [128, D], F32, kind="ExternalInput").ap()
        be2_d = nc.dram_tensor("be2d", [128, D], F32,
                               kind="ExternalInput").ap()
    out_d = nc.dram_tensor("out_q", [SQ, D], F32, kind="ExternalOutput").ap()
    dbg = bool(os.environ.get("KDBG"))
    if dbg:
        y1_d = nc.dram_tensor("y1_dbg", [SQ, D], F32,
                              kind="ExternalOutput").ap()

    w1v = w1_d.rearrange("p (c k) -> p c k", c=8)      # chunk jc -> (k2,j,512)
    w2v = w2_d.rearrange("p (c k) -> p c k", c=8)      # chunk i  -> (m2,jj,128)

    with tile.TileContext(nc) as tc:
        with (
            tc.tile_pool(name="const", bufs=1) as cpool,
            tc.tile_pool(name="resid", bufs=1) as rpool,
            tc.tile_pool(name="epi", bufs=1) as epool,
            tc.tile_pool(name="ffn", bufs=1) as fpool,
        ):
            bq_sb = cpool.tile([128, 8], F32)
            nc.sync.dma_start(bq_sb[:], bq_d[:])
            b1_sb = cpool.tile([128, 32], F32)
            nc.sync.dma_start(b1_sb[:], b1_d[:])
            b2_sb = cpool.tile([128, 8], F32)
            nc.sync.dma_start(b2_sb[:], b2_d[:])
            eps_sb = cpool.tile([128, 1], F32)
            nc.vector.memset(eps_sb[:], EPS)
            ones8 = cpool.tile([128, 2, 1], F8)
            nc.vector.memset(ones8[:], 1.0)
            ebias_sb = cpool.tile([128, 1], F32)
            nc.vector.memset(ebias_sb[:], EXP_BIAS)
            if apply_affine:
                g1_sb = cpool.tile([128, D], F32)
                nc.sync.dma_start(g1_sb[:], g1_d[:])
                be1_sb = cpool.tile([128, D], F32)
                nc.sync.dma_start(be1_sb[:], be1_d[:])
                g2_sb = cpool.tile([128, D], F32)
                nc.sync.dma_start(g2_sb[:], g2_d[:])
                be2_sb = cpool.tile([128, D], F32)
                nc.sync.dma_start(be2_sb[:], be2_d[:])

            # resident inputs
            xt2 = []
            for k2 in range(4):
                t = rpool.tile([128, 2, S], F8, tag=f"xt2_{k2}")
                nc.sync.dma_start(
                    t[:], xt2_d[:, k2 * 2 * S:(k2 + 1) * 2 * S]
                    .rearrange("p (j s) -> p j s", j=2))
                xt2.append(t)
            wq_sb = []
            for k2 in range(4):
                t = rpool.tile([128, 2, D], F8, tag=f"wq_{k2}")
                nc.sync.dma_start(
                    t[:], wq_d[:, k2 * 2 * D:(k2 + 1) * 2 * D]
                    .rearrange("p (j c) -> p j c", j=2))
                wq_sb.append(t)
            xq_sb = []
            for qt in range(4):
                t = rpool.tile([128, D], BF16, tag=f"xq_{qt}")
                nc.sync.dma_start(t[:], xq_d[qt * 128:(qt + 1) * 128, :])
                xq_sb.append(t)

            # epilogue / FFN tiles living across the whole kernel
            y1 = [epool.tile([128, D], F32, tag=f"y1_{qt}", name=f"y1_{qt}")
                  for qt in range(4)]
            x1bf = [epool.tile([128, D], BF16, tag=f"x1bf_{qt}",
                               name=f"x1bf_{qt}") for qt in range(4)]
            x1tstg = epool.tile([128, 4, 8, 128], BF16, tag="x1tstg")
            x1t2 = [epool.tile([128, 2, SQ], F8, tag=f"x1t2_{k2}",
                               name=f"x1t2_{k2}") for k2 in range(4)]
            h1 = [fpool.tile([128, 2, SQ], F8, tag=f"h1_{m2}",
                             name=f"h1_{m2}") for m2 in range(16)]
            ffb = fpool.tile([128, 4, D], BF16, tag="ffb")

            # ---- attention ----
            with (
                tc.tile_pool(name="qT", bufs=2) as qpoolb,
                tc.tile_pool(name="Ep", bufs=2) as epoolE,
                tc.tile_pool(name="att_s", bufs=2) as apool,
                tc.tile_pool(name="qps", bufs=2, space="PSUM") as qps,
                tc.tile_pool(name="scps", bufs=2, space="PSUM") as scps,
                tc.tile_pool(name="atps", bufs=2, space="PSUM") as atps,
            ):
                def emit_qproj(p):
                    """q projection for head pair p -> qt8 fp8 [128, S]."""
                    qt8 = qpoolb.tile([128, S], F8, tag="qt8",
                                      name=f"qt8_{p}")
                    for n in range(4):
                        ps = qps.tile([128, 512], F32, tag="qps",
                                      name=f"qps{p}_{n}")
                        for k2 in range(4):
                            nc.tensor.matmul(
                                ps[:],
                                wq_sb[k2][:, :, p * 128:(p + 1) * 128],
                                xt2[k2][:, :, n * 512:(n + 1) * 512],
                                start=(k2 == 0), stop=(k2 == 3),
                                perf_mode=MM_DR)
                        nc.vector.tensor_scalar_add(
                            qt8[:, n * 512:(n + 1) * 512], ps[:],
                            bq_sb[:, p:p + 1])
                    # fold for scores: [32,2]-partition layout per head
                    qtf = qpoolb.tile([64, 2, S], F8, tag="qtf",
                                      name=f"qtf_{p}")
                    for hh in range(2):
                        for jj in range(2):
                            r0 = hh * 64 + jj * 32
                            nc.sync.dma_start(
                                qtf[hh * 32:hh * 32 + 32, jj, :],
                                qt8[r0:r0 + 32, :])
                    # natural-layout q for the values path (bf16 -> fp8)
                    qtb = qpoolb.tile([128, S], BF16, tag="qtb",
                                      name=f"qtb_{p}")
                    for u in range(2):
                        sl = slice(u * 1024, (u + 1) * 1024)
                        nc.gpsimd.tensor_copy(qtb[:, sl], qt8[:, sl])
                    qastg = qpoolb.tile([128, 16, 128], BF16, tag="qastg",
                                        name=f"qastg_{p}")
                    nc.sync.dma_start(qastg[:], qtb[:], transpose=True)
                    qa8 = qpoolb.tile([128, 16, 128], F8, tag="qa8",
                                      name=f"qa8_{p}")
                    for u in range(2):
                        nc.gpsimd.tensor_copy(
                            qa8[:, u * 8:(u + 1) * 8, :],
                            qastg[:, u * 8:(u + 1) * 8, :])
                    return qtf, qa8

                def emit_scores_exp(p, qtf):
                    """scores + unnormalized exp -> E[g][hh] fp8 [128, 1024]."""
                    E = [[None, None] for _ in range(8)]
                    for g in range(8):
                        for hh in range(2):
                            sc = scps.tile([128, 1024], F32, tag="sc",
                                           name=f"sc{p}_{g}_{hh}")
                            for kb2 in range(2):
                                kb = 2 * g + kb2
                                nc.tensor.matmul(
                                    sc[:, kb2 * 512:(kb2 + 1) * 512],
                                    qtf[hh * 32:hh * 32 + 32, :,
                                        kb * 128:(kb + 1) * 128],
                                    qtf[hh * 32:hh * 32 + 32, :, 0:512],
                                    start=True, stop=True, perf_mode=MM_DR)
                            Eg = epoolE.tile([128, 1024], F8,
                                             tag=f"E{g}_{hh}",
                                             name=f"E{p}_{g}_{hh}")
                            if (g, hh) in EXP_DVE_UNITS:
                                # uint8 out: device saturates negatives
                                # to 0 (= fp8 +0) so the far-negative score
                                # tail clips instead of wrapping to -NaN
                                ei = nc.vector.tensor_scalar(
                                    Eg[:].bitcast(U8), sc[:],
                                    LOG2E, EXP_B8, ALU.mult, ALU.add)
                            else:
                                ei = nc.scalar.activation(
                                    Eg[:], sc[:], AF.Exp,
                                    bias=ebias_sb[:, 0:1], scale=0.125)
                            E[g][hh] = (Eg, ei)
                    return E

                def emit_wv(p, E, qa8):
                    """attention values + denominators + y1 epilogue."""
                    for qt in range(4):
                        att = atps.tile([128, 512], F32, tag="att",
                                        name=f"att{p}_{qt}")
                        for g in range(8):
                            for hh in range(2):
                                Eg, ei = E[g][hh]
                                Es = (Eg[:]
                                      .rearrange("k (b q) -> k b q", b=2)
                                      [:, :, qt * 128:(qt + 1) * 128])
                                mm = nc.tensor.matmul(
                                    att[:, hh * 65:hh * 65 + 64], Es,
                                    qa8[:, 2 * g:2 * g + 2,
                                        hh * 64:(hh + 1) * 64],
                                    start=(g == 0 and hh == 0), stop=False,
                                    perf_mode=MM_DR, skip_group_check=True)
                                if qt == 0 and (g, hh) in EXP_DVE_UNITS:
                                    add_dep_helper(
                                        mm.ins, ei.ins,
                                        reason="E bitcast write -> PE read")
                                nc.tensor.matmul(
                                    att[:, hh * 65 + 64:hh * 65 + 65], Es,
                                    ones8[:],
                                    start=False, stop=(g == 7 and hh == 1),
                                    perf_mode=MM_DR, skip_group_check=True)
                        dn = apool.tile([128, 2, 1], F32, tag="dn",
                                        name=f"dn{p}_{qt}")
                        nc.vector.tensor_copy(
                            dn[:], att[:, 0:130]
                            .rearrange("a (h c) -> a h c", h=2)[:, :, 64:65])
                        rct = apool.tile([128, 2, 1], F32, tag="rct",
                                         name=f"rct{p}_{qt}")
                        nc.vector.reciprocal(rct[:], dn[:])
                        for hh in range(2):
                            h = 2 * p + hh
                            sl = slice(h * 64, (h + 1) * 64)
                            nc.vector.scalar_tensor_tensor(
                                y1[qt][:, sl],
                                att[:, hh * 65:hh * 65 + 64],
                                rct[:, hh, :],
                                xq_sb[qt][:, sl], ALU.mult, ALU.add)

                qtf, qa8 = emit_qproj(0)
                for p in range(8):
                    E = emit_scores_exp(p, qtf)
                    if p < 7:
                        nqtf, nqa8 = emit_qproj(p + 1)
                    emit_wv(p, E, qa8)
                    if p < 7:
                        qtf, qa8 = nqtf, nqa8

            if dbg:
                for qt in range(4):
                    nc.sync.dma_start(y1_d[qt * 128:(qt + 1) * 128, :],
                                      y1[qt][:])

            # ---- LN1 + x1 transpose ----
            for qt in range(4):
                x1 = _layer_norm(nc, epool, y1[qt], x1bf[qt], eps_sb,
                                 (g1_sb, be1_sb) if apply_affine else None)
                nc.sync.dma_start(x1tstg[:, qt, :, :], x1bf[qt][:],
                                  transpose=True)
                for k2 in range(4):
                    nc.vector.tensor_copy(
                        x1t2[k2][:, :, qt * 128:(qt + 1) * 128],
                        x1tstg[:, qt, 2 * k2:2 * k2 + 2, :])

            # ---- FFN ----
            with (
                tc.tile_pool(name="wstr", bufs=3) as wpool,
                tc.tile_pool(name="fps", bufs=4, space="PSUM") as fps,
            ):
                for jc in range(8):
                    w1c = wpool.tile([128, 4, 2, 512], F8, tag="w1c",
                                     name=f"w1c_{jc}")
                    nc.sync.dma_start(
                        w1c[:], w1v[:, jc, :]
                        .rearrange("p (a b k) -> p a b k", a=4, b=2))
                    for jj4 in range(4):
                        j = jc * 4 + jj4
                        ps = fps.tile([128, 512], F32, tag="fps",
                                      name=f"f1ps_{j}")
                        for k2 in range(4):
                            nc.tensor.matmul(
                                ps[:],
                                w1c[:, k2, :, jj4 * 128:(jj4 + 1) * 128],
                                x1t2[k2][:], start=(k2 == 0), stop=(k2 == 3),
                                perf_mode=MM_DR)
                        if j % 2 == 0:
                            nc.scalar.activation(
                                h1[j // 2][:, j % 2, :], ps[:], AF.Relu,
                                bias=b1_sb[:, j:j + 1])
                        else:
                            nc.vector.tensor_scalar(
                                h1[j // 2][:, j % 2, :], ps[:],
                                b1_sb[:, j:j + 1], 0.0, ALU.add, ALU.max)
                for i in range(8):
                    w2c = wpool.tile([128, 16, 2, 128], F8, tag="w2c",
                                     name=f"w2c_{i}")
                    nc.sync.dma_start(
                        w2c[:], w2v[:, i, :]
                        .rearrange("p (a b k) -> p a b k", a=16, b=2))
                    ps = fps.tile([128, 512], F32, tag="fps",
                                  name=f"f2ps_{i}")
                    for m2 in range(16):
                        nc.tensor.matmul(
                            ps[:], w2c[:, m2, :, :], h1[m2][:],
                            start=(m2 == 0), stop=(m2 == 15),
                            perf_mode=MM_DR)
                    fft = wpool.tile([128, 512], BF16, tag="fft", bufs=4,
                                     name=f"fft_{i}")
                    nc.vector.tensor_scalar(
                        fft[:], ps[:], FFS, b2_sb[:, i:i + 1],
                        ALU.mult, ALU.add)
                    nc.sync.dma_start(ffb[:, :, i * 128:(i + 1) * 128],
                                      fft[:], transpose=True)

            # ---- residual + LN2 + output ----
            for qt in range(4):
                y2 = epool.tile([128, D], F32, tag=f"y1_{qt}",
                                name=f"y2_{qt}")
                nc.gpsimd.tensor_add(y2[:], x1bf[qt][:], ffb[:, qt, :])
                x2 = epool.tile([128, D], F32, tag="x2", bufs=2,
                                name=f"x2_{qt}")
                _layer_norm(nc, epool, y2, x2, eps_sb,
                            (g2_sb, be2_sb) if apply_affine else None)
                nc.sync.dma_start(out_d[qt * 128:(qt + 1) * 128, :], x2[:])

    nc.compile()
    _BUILD_CACHE[apply_affine] = nc
    return nc


def _layer_norm(nc, pool, y, out, eps_sb, affine):
    """out = (y - mean(y)) * rsqrt(var(y) + EPS) [* g + b], free-dim D."""
    s1 = pool.tile([128, 1], F32, tag="ln_s1")
    nc.vector.reduce_sum(s1[:], y[:], axis=mybir.AxisListType.X)
    mean = pool.tile([128, 1], F32, tag="ln_mean")
    nc.vector.tensor_scalar_mul(mean[:], s1[:], 1.0 / D)
    cen = pool.tile([128, D], F32, tag="ln_cen", bufs=2)
    nc.vector.tensor_scalar_sub(cen[:], y[:], mean[:])
    scr = pool.tile([128, D], F32, tag="ln_scr", bufs=2)
    var = pool.tile([128, 1], F32, tag="ln_var")
    nc.scalar.activation(scr[:], cen[:], AF.Square, accum_out=var[:])
    std = pool.tile([128, 1], F32, tag="ln_std")
    nc.scalar.activation(std[:], var[:], AF.Sqrt, bias=eps_sb[:, 0:1],
                         scale=1.0 / D)
    rstd = pool.tile([128, 1], F32, tag="ln_rstd")
    nc.vector.reciprocal(rstd[:], std[:])
    if affine is None:
        nc.vector.tensor_scalar_mul(out[:], cen[:], rstd[:])
    else:
        g_sb, b_sb = affine
        nc.vector.scalar_tensor_tensor(
            out[:], cen[:], rstd[:], g_sb[:], ALU.mult, ALU.mult)
        nc.vector.tensor_add(out[:], out[:], b_sb[:])
    return out


def _prep_static(Wq, bq, W1, b1, W2, b2):
    f8 = dt.np(F8)
    wq_flat = np.asarray(Wq, np.float32).transpose(1, 0, 2).reshape(D, D)
    return {
        "wq2": np.ascontiguousarray(
            wq_flat.reshape(4, 2, 128, D).transpose(2, 0, 1, 3)
            .reshape(128, 8 * D)).astype(f8),
        "bqr": np.ascontiguousarray(
            np.asarray(bq, np.float32).reshape(8, 128).T),
        "w1p": np.ascontiguousarray(
            (np.asarray(W1, np.float32) * W1S)
            .reshape(4, 2, 128, 8, 512).transpose(2, 3, 0, 1, 4)
            .reshape(128, 8 * 4096)).astype(f8),
        "b1r": np.ascontiguousarray(
            (np.asarray(b1, np.float32) * W1S).reshape(32, 128).T),
        "w2p": np.ascontiguousarray(
            (np.asarray(W2, np.float32) * W2S)
            .reshape(16, 2, 128, 8, 128).transpose(2, 3, 0, 1, 4)
            .reshape(128, 8 * 4096)).astype(f8),
        "b2r": np.ascontiguousarray(
            np.asarray(b2, np.float32).reshape(8, 128).T),
    }


def kernel(x, Wq, bq, ln1_g, ln1_b, W1, b1, W2, b2, ln2_g, ln2_b):
    x = np.asarray(x, np.float32)
    f8 = dt.np(F8)
    bf = dt.np(BF16)
    trivial = (np.all(ln1_g == 1) and np.all(ln1_b == 0)
               and np.all(ln2_g == 1) and np.all(ln2_b == 0))
    nc = _build(apply_affine=not trivial)

    base = _prep_static(Wq, bq, W1, b1, W2, b2)
    if not trivial:
        for name, v in (("g1d", ln1_g), ("be1d", ln1_b),
                        ("g2d", ln2_g), ("be2d", ln2_b)):
            base[name] = np.ascontiguousarray(
                np.broadcast_to(np.asarray(v, np.float32), (128, D)))

    in_maps = []
    for c in range(NCORES):
        b, t = divmod(c, 4)
        xb = np.concatenate([x[b, t * SQ:], x[b, :t * SQ]], axis=0)
        xT = np.ascontiguousarray(xb.T)                     # [D, S]
        in_maps.append({
            **base,
            "xt2": np.ascontiguousarray(
                xT.reshape(4, 2, 128, S).transpose(2, 0, 1, 3)
                .reshape(128, 8 * S)).astype(f8),
            "xq": np.ascontiguousarray(xb[0:SQ]).astype(bf),
        })

    import os
    trace = bool(int(os.environ.get("KERNEL_TRACE", "0")))
    kw = {}
    if trace:
        kw = dict(trace=True,
                  tmpdir=os.environ.get("KERNEL_TRACE_DIR") or None)
    res = run_bass_kernel_spmd(nc, in_maps, core_ids=list(range(NCORES)),
                               **kw)
    if trace:
        print(f"HW exec time: {res.exec_time_ns} ns  "
              f"(mean {res.mean_exec_time_ns}, "
              f"max core {res.max_exec_time_core_id})")
    out = np.empty((B, S, D), np.float32)
    for c in range(NCORES):
        b, t = divmod(c, 4)
        out[b, t * SQ:(t + 1) * SQ] = res.results[c]["out_q"]
    return out


# revision 18
# speedup vs baseline: 1.7518x; 1.0531x over previous
"""Trainium2 Bass kernel for a dense transformer encoder layer.

Reference semantics (B=2, S=2048, D=1024, H=16, DH=64, HID=4096):
    q = einsum('bsd,hde->bhse', x, Wq) + bq          (q == k == v, source bug)
    prob = softmax(q @ q^T / sqrt(DH))
    attn = concat_heads(prob @ q)
    x1 = LN(x + attn);  ff = relu(x1 @ W1 + b1) @ W2 + b2;  out = LN(x1 + ff)

Sharding: 8 cores, core c -> batch b=c//4, token quarter t=c%4.  Each core
computes q for the full sequence of its batch (replicated inside the 4-core
group -> zero collectives), then attention + FFN for its own 512 tokens.
The host rotates each core's copy of x[b] so its quarter lands at rows 0:512
(attention is permutation-equivariant over keys), and reassembles quarters.

Precision strategy: all five matmul groups run as fp8(e4m3) DoubleRow
(qproj / scores / wv / FFN-w1 / FFN-w2), accumulating in f32 PSUM.  W1 is
pre-scaled by 32 and W2 by 64 so their entries sit in e4m3's normal range;
the combined 1/2048 is folded into the final bias-add.  Softmax weights are
unnormalized E = exp(s/8 - 2) stored in fp8: the ACT engine computes exact
exp for its share of tiles while the DVE computes a Schraudolph-style
bit-construction (s*log2e + B rounded to int8 IS the e4m3 bit pattern of
~exp(s/8-2)); denominators come from an extra ones-column matmul and divide
through after the fact, so the uniform scale/approximation bias cancels.

Layout tricks: the host uploads x already transposed and fold-paired for
DoubleRow, so the only on-device transposes are batched 16x128-tile XBAR
DMA transposes (qa, x1T, ffb) -- each one instruction for a whole [128,
2048] operand, keeping the HWDGE issue count (~625ns each, exclusive) low.
Scores contract over DH=64 folded to [32,2] partitions via 4 SBUF->SBUF
partition-moving DMAs per head pair.
"""

import numpy as np

import concourse.bacc as bacc
import concourse.mybir as mybir
from concourse import tile
from concourse.bass_utils import run_bass_kernel_spmd
from concourse.tile_rust import add_dep_helper

dt = mybir.dt
AF = mybir.ActivationFunctionType
ALU = mybir.AluOpType
MM_DR = mybir.MatmulPerfMode.DoubleRow

B, S, D = 2, 2048, 1024
H, DH, HID = 16, 64, 4096
SQ = S // 4            # tokens per core
NCORES = 8
EPS = 1e-5
F32, BF16, F8, U8 = dt.float32, dt.bfloat16, dt.float8e4, dt.uint8

W1S, W2S = 32.0, 64.0          # host-side fp8 weight scales
FFS = 1.0 / (W1S * W2S)        # folded back in the last FFN bias-add
LOG2E = 1.4426950408889634
EXP_B8 = 32.58                 # e4m3 bit bias: 55.66 - 8*2*log2(e), so the
                               # bit-trick matches the ACT tiles' exp(s/8-2)
EXP_BIAS = -2.0                # matching shift on the exact-exp (ACT) tiles

# exp units (g, hh) handed to the DVE bit-trick; the rest go to ACT exact
# exp.  Interleaved through the pair so ACT and DVE run concurrently.
import os
EXP_DVE_UNITS = (set() if os.environ.get("NO_DVE_EXP") else
                 {(0, 1), (2, 0), (3, 1), (5, 0), (6, 1)})

_BUILD_CACHE = {}


def _build(apply_affine: bool):
    if apply_affine in _BUILD_CACHE:
        return _BUILD_CACHE[apply_affine]

    nc = bacc.Bacc("TRN2", target_bir_lowering=False, debug=False,
                   num_devices=NCORES)

    xt2_d = nc.dram_tensor("xt2", [128, 4 * 2 * S], F8,
                           kind="ExternalInput").ap()
    xq_d = nc.dram_tensor("xq", [SQ, D], BF16, kind="ExternalInput").ap()
    wq_d = nc.dram_tensor("wq2", [128, 4 * 2 * D], F8,
                          kind="ExternalInput").ap()
    bq_d = nc.dram_tensor("bqr", [128, 8], F32, kind="ExternalInput").ap()
    w1_d = nc.dram_tensor("w1p", [128, 8 * 4096], F8,
                          kind="ExternalInput").ap()
    b1_d = nc.dram_tensor("b1r", [128, 32], F32, kind="ExternalInput").ap()
    w2_d = nc.dram_tensor("w2p", [128, 8 * 4096], F8,
                          kind="ExternalInput").ap()
    b2_d = nc.dram_tensor("b2r", [128, 8], F32, kind="ExternalInput").ap()
    if apply_affine:
        g1_d = nc.dram_tensor("g1d", [128, D], F32, kind="ExternalInput").ap()
        be1_d = nc.dram_tensor("be1d", [128, D], F32,
                               kind="ExternalInput").ap()
        g2_d = nc.dram_tensor("g2d", [128, D], F32, kind="ExternalInput").ap()
        be2_d = nc.dram_tensor("be2d", [128, D], F32,
                               kind="ExternalInput").ap()
    out_d = nc.dram_tensor("out_q", [SQ, D], F32, kind="ExternalOutput").ap()
    dbg = bool(os.environ.get("KDBG"))
    if dbg:
        y1_d = nc.dram_tensor("y1_dbg", [SQ, D], F32,
                              kind="ExternalOutput").ap()

    w1v = w1_d.rearrange("p (c k) -> p c k", c=8)      # chunk jc -> (k2,j,512)
    w2v = w2_d.rearrange("p (c k) -> p c k", c=8)      # chunk i  -> (m2,jj,128)

    with tile.TileContext(nc) as tc:
        with (
            tc.tile_pool(name="const", bufs=1) as cpool,
            tc.tile_pool(name="resid", bufs=1) as rpool,
            tc.tile_pool(name="epi", bufs=1) as epool,
            tc.tile_pool(name="ffn", bufs=1) as fpool,
        ):
            bq_sb = cpool.tile([128, 8], F32)
            nc.sync.dma_start(bq_sb[:], bq_d[:])
            b1_sb = cpool.tile([128, 32], F32)
            nc.sync.dma_start(b1_sb[:], b1_d[:])
            b2_sb = cpool.tile([128, 8], F32)
            nc.sync.dma_start(b2_sb[:], b2_d[:])
            eps_sb = cpool.tile([128, 1], F32)
            nc.vector.memset(eps_sb[:], EPS)
            ones8 = cpool.tile([128, 2, 1], F8)
            nc.vector.memset(ones8[:], 1.0)
            ebias_sb = cpool.tile([128, 1], F32)
            nc.vector.memset(ebias_sb[:], EXP_BIAS)
            if apply_affine:
                g1_sb = cpool.tile([128, D], F32)
                nc.sync.dma_start(g1_sb[:], g1_d[:])
                be1_sb = cpool.tile([128, D], F32)
                nc.sync.dma_start(be1_sb[:], be1_d[:])
                g2_sb = cpool.tile([128, D], F32)
                nc.sync.dma_start(g2_sb[:], g2_d[:])
                be2_sb = cpool.tile([128, D], F32)
                nc.sync.dma_start(be2_sb[:], be2_d[:])

            # resident inputs
            xt2 = []
            for k2 in range(4):
                t = rpool.tile([128, 2, S], F8, tag=f"xt2_{k2}")
                nc.sync.dma_start(
                    t[:], xt2_d[:, k2 * 2 * S:(k2 + 1) * 2 * S]
                    .rearrange("p (j s) -> p j s", j=2))
                xt2.append(t)
            wq_sb = []
            for k2 in range(4):
                t = rpool.tile([128, 2, D], F8, tag=f"wq_{k2}")
                nc.sync.dma_start(
                    t[:], wq_d[:, k2 * 2 * D:(k2 + 1) * 2 * D]
                    .rearrange("p (j c) -> p j c", j=2))
                wq_sb.append(t)
            xq_sb = []
            for qt in range(4):
                t = rpool.tile([128, D], BF16, tag=f"xq_{qt}")
                nc.sync.dma_start(t[:], xq_d[qt * 128:(qt + 1) * 128, :])
                xq_sb.append(t)

            # epilogue / FFN tiles living across the whole kernel
            y1 = [epool.tile([128, D], F32, tag=f"y1_{qt}", name=f"y1_{qt}")
                  for qt in range(4)]
            x1bf = [epool.tile([128, D], BF16, tag=f"x1bf_{qt}",
                               name=f"x1bf_{qt}") for qt in range(4)]
            x1tstg = epool.tile([128, 4, 8, 128], BF16, tag="x1tstg")
            x1t2 = [epool.tile([128, 2, SQ], F8, tag=f"x1t2_{k2}",
                               name=f"x1t2_{k2}") for k2 in range(4)]
            h1 = [fpool.tile([128, 2, SQ], F8, tag=f"h1_{m2}",
                             name=f"h1_{m2}") for m2 in range(16)]
            ffb = fpool.tile([128, 4, D], BF16, tag="ffb")

            # ---- attention ----
            with (
                tc.tile_pool(name="qT", bufs=2) as qpoolb,
                tc.tile_pool(name="Ep", bufs=2) as epoolE,
                tc.tile_pool(name="att_s", bufs=2) as apool,
                tc.tile_pool(name="qps", bufs=1, space="PSUM") as qps,
                tc.tile_pool(name="scps", bufs=3, space="PSUM") as scps,
                tc.tile_pool(name="atps", bufs=1, space="PSUM") as atps,
            ):
                def emit_qproj(p):
                    """q projection for head pair p -> qt8 fp8 [128, S]."""
                    qt8 = qpoolb.tile([128, S], F8, tag="qt8",
                                      name=f"qt8_{p}")
                    for n in range(4):
                        ps = qps.tile([128, 512], F32, tag="qps",
                                      name=f"qps{p}_{n}")
                        for k2 in range(4):
                            nc.tensor.matmul(
                                ps[:],
                                wq_sb[k2][:, :, p * 128:(p + 1) * 128],
                                xt2[k2][:, :, n * 512:(n + 1) * 512],
                                start=(k2 == 0), stop=(k2 == 3),
                                perf_mode=MM_DR)
                        nc.vector.tensor_scalar_add(
                            qt8[:, n * 512:(n + 1) * 512], ps[:],
                            bq_sb[:, p:p + 1])
                    # fold for scores: [32,2]-partition layout per head
                    qtf = qpoolb.tile([64, 2, S], F8, tag="qtf",
                                      name=f"qtf_{p}")
                    for hh in range(2):
                        for jj in range(2):
                            r0 = hh * 64 + jj * 32
                            nc.sync.dma_start(
                                qtf[hh * 32:hh * 32 + 32, jj, :],
                                qt8[r0:r0 + 32, :])
                    # natural-layout q for the values path (bf16 -> fp8)
                    qtb = qpoolb.tile([128, S], BF16, tag="qtb",
                                      name=f"qtb_{p}")
                    for u in range(2):
                        sl = slice(u * 1024, (u + 1) * 1024)
                        nc.gpsimd.tensor_copy(qtb[:, sl], qt8[:, sl])
                    qastg = qpoolb.tile([128, 16, 128], BF16, tag="qastg",
                                        name=f"qastg_{p}")
                    nc.sync.dma_start(qastg[:], qtb[:], transpose=True)
                    qa8 = qpoolb.tile([128, 16, 128], F8, tag="qa8",
                                      name=f"qa8_{p}")
                    for u in range(2):
                        nc.gpsimd.tensor_copy(
                            qa8[:, u * 8:(u + 1) * 8, :],
                            qastg[:, u * 8:(u + 1) * 8, :])
                    return qtf, qa8

                def emit_scores_exp(p, qtf):
                    """scores + unnormalized exp -> E[g][hh] fp8 [128, 1024]."""
                    E = [[None, None] for _ in range(8)]
                    for g in range(8):
                        for hh in range(2):
                            sc = scps.tile([128, 1024], F32, tag="sc",
                                           name=f"sc{p}_{g}_{hh}")
                            for kb2 in range(2):
                                kb = 2 * g + kb2
                                nc.tensor.matmul(
                                    sc[:, kb2 * 512:(kb2 + 1) * 512],
                                    qtf[hh * 32:hh * 32 + 32, :,
                                        kb * 128:(kb + 1) * 128],
                                    qtf[hh * 32:hh * 32 + 32, :, 0:512],
                                    start=True, stop=True, perf_mode=MM_DR)
                            Eg = epoolE.tile([128, 1024], F8,
                                             tag=f"E{g}_{hh}",
                                             name=f"E{p}_{g}_{hh}")
                            if (g, hh) in EXP_DVE_UNITS:
                                # uint8 out: device saturates negatives
                                # to 0 (= fp8 +0) so the far-negative score
                                # tail clips instead of wrapping to -NaN
                                ei = nc.vector.tensor_scalar(
                                    Eg[:].bitcast(U8), sc[:],
                                    LOG2E, EXP_B8, ALU.mult, ALU.add)
                            else:
                                ei = nc.scalar.activation(
                                    Eg[:], sc[:], AF.Exp,
                                    bias=ebias_sb[:, 0:1], scale=0.125)
                            E[g][hh] = (Eg, ei)
                    return E

                def emit_wv(p, E, qa8):
                    """attention values + denominators + y1 epilogue."""
                    for qt in range(4):
                        att = atps.tile([128, 512], F32, tag="att",
                                        name=f"att{p}_{qt}")
                        for g in range(8):
                            for hh in range(2):
                                Eg, ei = E[g][hh]
                                Es = (Eg[:]
                                      .rearrange("k (b q) -> k b q", b=2)
                                      [:, :, qt * 128:(qt + 1) * 128])
                                mm = nc.tensor.matmul(
                                    att[:, hh * 65:hh * 65 + 64], Es,
                                    qa8[:, 2 * g:2 * g + 2,
                                        hh * 64:(hh + 1) * 64],
                                    start=(g == 0 and hh == 0), stop=False,
                                    perf_mode=MM_DR, skip_group_check=True)
                                if qt == 0 and (g, hh) in EXP_DVE_UNITS:
                                    add_dep_helper(
                                        mm.ins, ei.ins,
                                        reason="E bitcast write -> PE read")
                                nc.tensor.matmul(
                                    att[:, hh * 65 + 64:hh * 65 + 65], Es,
                                    ones8[:],
                                    start=False, stop=(g == 7 and hh == 1),
                                    perf_mode=MM_DR, skip_group_check=True)
                        dn = apool.tile([128, 2, 1], F32, tag="dn",
                                        name=f"dn{p}_{qt}")
                        nc.vector.tensor_copy(
                            dn[:], att[:, 0:130]
                            .rearrange("a (h c) -> a h c", h=2)[:, :, 64:65])
                        rct = apool.tile([128, 2, 1], F32, tag="rct",
                                         name=f"rct{p}_{qt}")
                        nc.vector.reciprocal(rct[:], dn[:])
                        for hh in range(2):
                            h = 2 * p + hh
                            sl = slice(h * 64, (h + 1) * 64)
                            nc.vector.scalar_tensor_tensor(
                                y1[qt][:, sl],
                                att[:, hh * 65:hh * 65 + 64],
                                rct[:, hh, :],
                                xq_sb[qt][:, sl], ALU.mult, ALU.add)

                qtf, qa8 = emit_qproj(0)
                for p in range(8):
                    E = emit_scores_exp(p, qtf)
                    if p < 7:
                        nqtf, nqa8 = emit_qproj(p + 1)
                    emit_wv(p, E, qa8)
                    if p < 7:
                        qtf, qa8 = nqtf, nqa8

            if dbg:
                for qt in range(4):
                    nc.sync.dma_start(y1_d[qt * 128:(qt + 1) * 128, :],
                                      y1[qt][:])

            # ---- LN1 + x1 transpose ----
            for qt in range(4):
                x1 = _layer_norm(nc, epool, y1[qt], x1bf[qt], eps_sb,
                                 (g1_sb, be1_sb) if apply_affine else None)
                nc.scalar.dma_start(x1tstg[:, qt, :, :], x1bf[qt][:],
                                    transpose=True)
                for k2 in range(4):
                    nc.vector.tensor_copy(
                        x1t2[k2][:, :, qt * 128:(qt + 1) * 128],
                        x1tstg[:, qt, 2 * k2:2 * k2 + 2, :])

            # ---- FFN ----
            with (
                tc.tile_pool(name="wstr", bufs=3) as wpool,
                tc.tile_pool(name="fps", bufs=4, space="PSUM") as fps,
            ):
                for jc in range(8):
                    w1c = wpool.tile([128, 4, 2, 512], F8, tag="w1c",
                                     name=f"w1c_{jc}")
                    nc.sync.dma_start(
                        w1c[:], w1v[:, jc, :]
                        .rearrange("p (a b k) -> p a b k", a=4, b=2))
                    for jj4 in range(4):
                        j = jc * 4 + jj4
                        ps = fps.tile([128, 512], F32, tag="fps",
                                      name=f"f1ps_{j}")
                        for k2 in range(4):
                            nc.tensor.matmul(
                                ps[:],
                                w1c[:, k2, :, jj4 * 128:(jj4 + 1) * 128],
                                x1t2[k2][:], start=(k2 == 0), stop=(k2 == 3),
                                perf_mode=MM_DR)
                        if j % 2 == 0:
                            nc.scalar.activation(
                                h1[j // 2][:, j % 2, :], ps[:], AF.Relu,
                                bias=b1_sb[:, j:j + 1])
                        else:
                            nc.vector.tensor_scalar(
                                h1[j // 2][:, j % 2, :], ps[:],
                                b1_sb[:, j:j + 1], 0.0, ALU.add, ALU.max)
                for i in range(8):
                    w2c = wpool.tile([128, 16, 2, 128], F8, tag="w2c",
                                     name=f"w2c_{i}")
                    nc.sync.dma_start(
                        w2c[:], w2v[:, i, :]
                        .rearrange("p (a b k) -> p a b k", a=16, b=2))
                    ps = fps.tile([128, 512], F32, tag="fps",
                                  name=f"f2ps_{i}")
                    for m2 in range(16):
                        nc.tensor.matmul(
                            ps[:], w2c[:, m2, :, :], h1[m2][:],
                            start=(m2 == 0), stop=(m2 == 15),
                            perf_mode=MM_DR)
                    fft = wpool.tile([128, 512], BF16, tag="fft", bufs=4,
                                     name=f"fft_{i}")
                    nc.vector.tensor_scalar(
                        fft[:], ps[:], FFS, b2_sb[:, i:i + 1],
                        ALU.mult, ALU.add)
                    nc.scalar.dma_start(ffb[:, :, i * 128:(i + 1) * 128],
                                        fft[:], transpose=True)

            # ---- residual + LN2 + output ----
            for qt in range(4):
                y2 = epool.tile([128, D], F32, tag=f"y1_{qt}",
                                name=f"y2_{qt}")
                nc.gpsimd.tensor_add(y2[:], x1bf[qt][:], ffb[:, qt, :])
                x2 = epool.tile([128, D], F32, tag="x2", bufs=2,
                                name=f"x2_{qt}")
                _layer_norm(nc, epool, y2, x2, eps_sb,
                            (g2_sb, be2_sb) if apply_affine else None)
                nc.sync.dma_start(out_d[qt * 128:(qt + 1) * 128, :], x2[:])

    nc.compile()
    _BUILD_CACHE[apply_affine] = nc
    return nc


def _layer_norm(nc, pool, y, out, eps_sb, affine):
    """out = (y - mean(y)) * rsqrt(var(y) + EPS) [* g + b], free-dim D."""
    s1 = pool.tile([128, 1], F32, tag="ln_s1")
    nc.vector.reduce_sum(s1[:], y[:], axis=mybir.AxisListType.X)
    mean = pool.tile([128, 1], F32, tag="ln_mean")
    nc.vector.tensor_scalar_mul(mean[:], s1[:], 1.0 / D)
    cen = pool.tile([128, D], F32, tag="ln_cen", bufs=2)
    nc.vector.tensor_scalar_sub(cen[:], y[:], mean[:])
    scr = pool.tile([128, D], F32, tag="ln_scr", bufs=2)
    var = pool.tile([128, 1], F32, tag="ln_var")
    nc.scalar.activation(scr[:], cen[:], AF.Square, accum_out=var[:])
    std = pool.tile([128, 1], F32, tag="ln_std")
    nc.scalar.activation(std[:], var[:], AF.Sqrt, bias=eps_sb[:, 0:1],
                         scale=1.0 / D)
    rstd = pool.tile([128, 1], F32, tag="ln_rstd")
    nc.vector.reciprocal(rstd[:], std[:])
    if affine is None:
        nc.vector.tensor_scalar_mul(out[:], cen[:], rstd[:])
    else:
        g_sb, b_sb = affine
        nc.vector.scalar_tensor_tensor(
            out[:], cen[:], rstd[:], g_sb[:], ALU.mult, ALU.mult)
        nc.vector.tensor_add(out[:], out[:], b_sb[:])
    return out


def _prep_static(Wq, bq, W1, b1, W2, b2):
    f8 = dt.np(F8)
    wq_flat = np.asarray(Wq, np.float32).transpose(1, 0, 2).reshape(D, D)
    return {
        "wq2": np.ascontiguousarray(
            wq_flat.reshape(4, 2, 128, D).transpose(2, 0, 1, 3)
            .reshape(128, 8 * D)).astype(f8),
        "bqr": np.ascontiguousarray(
            np.asarray(bq, np.float32).reshape(8, 128).T),
        "w1p": np.ascontiguousarray(
            (np.asarray(W1, np.float32) * W1S)
            .reshape(4, 2, 128, 8, 512).transpose(2, 3, 0, 1, 4)
            .reshape(128, 8 * 4096)).astype(f8),
        "b1r": np.ascontiguousarray(
            (np.asarray(b1, np.float32) * W1S).reshape(32, 128).T),
        "w2p": np.ascontiguousarray(
            (np.asarray(W2, np.float32) * W2S)
            .reshape(16, 2, 128, 8, 128).transpose(2, 3, 0, 1, 4)
            .reshape(128, 8 * 4096)).astype(f8),
        "b2r": np.ascontiguousarray(
            np.asarray(b2, np.float32).reshape(8, 128).T),
    }


def kernel(x, Wq, bq, ln1_g, ln1_b, W1, b1, W2, b2, ln2_g, ln2_b):
    x = np.asarray(x, np.float32)
    f8 = dt.np(F8)
    bf = dt.np(BF16)
    trivial = (np.all(ln1_g == 1) and np.all(ln1_b == 0)
               and np.all(ln2_g == 1) and np.all(ln2_b == 0))
    nc = _build(apply_affine=not trivial)

    base = _prep_static(Wq, bq, W1, b1, W2, b2)
    if not trivial:
        for name, v in (("g1d", ln1_g), ("be1d", ln1_b),
                        ("g2d", ln2_g), ("be2d", ln2_b)):
            base[name] = np.ascontiguousarray(
                np.broadcast_to(np.asarray(v, np.float32), (128, D)))

    in_maps = []
    for c in range(NCORES):
        b, t = divmod(c, 4)
        xb = np.concatenate([x[b, t * SQ:], x[b, :t * SQ]], axis=0)
        xT = np.ascontiguousarray(xb.T)                     # [D, S]
        in_maps.append({
            **base,
            "xt2": np.ascontiguousarray(
                xT.reshape(4, 2, 128, S).transpose(2, 0, 1, 3)
                .reshape(128, 8 * S)).astype(f8),
            "xq": np.ascontiguousarray(xb[0:SQ]).astype(bf),
        })

    import os
    trace = bool(int(os.environ.get("KERNEL_TRACE", "0")))
    kw = {}
    if trace:
        kw = dict(trace=True,
                  tmpdir=os.environ.get("KERNEL_TRACE_DIR") or None)
    res = run_bass_kernel_spmd(nc, in_maps, core_ids=list(range(NCORES)),
                               **kw)
    if trace:
        print(f"HW exec time: {res.exec_time_ns} ns  "
              f"(mean {res.mean_exec_time_ns}, "
              f"max core {res.max_exec_time_core_id})")
    out = np.empty((B, S, D), np.float32)
    for c in range(NCORES):
        b, t = divmod(c, 4)
        out[b, t * SQ:(t + 1) * SQ] = res.results[c]["out_q"]
    return out
